# revision 9
# baseline (speedup 1.0000x reference)
"""BiasedMHA Trainium2 kernel.

Problem: B=4, N=1024, FEAT=512, H=8 multihead attention with additive bias and
boolean mask, softmax over the key dim, output projection.

Sharding (8 cores): core c handles batch b = c//2 and query-row half
ih = c%2 (512 query rows). Each core computes the full attention + output
projection for its slice -> out[b, ih*512:(ih+1)*512, :]. No collectives.

Per-core device layout (everything pre-transposed on host so DMAs are
contiguous and matmuls contract along partitions):
  - QT[d, i], KT[d, j] (head-major d), V'[j, 8*(64+1)] with a ones column per
    head (gives the softmax denominator for free during the PV matmul).
  - scores computed transposed: ST[j-chunk, i] = K_h^T(chunk) x Q_h
  - bias (with mask folded in as -1e30) is added with the DVE, exp on ACT.
  - PV: out'[65, i] accumulated over j-chunks; row 64 = sum of exp = denom.
  - normalization: recip = 1/denom, broadcast along partitions via a K=1
    matmul, applied with one tensor-tensor multiply; the output projection
    then runs on normalized head outputs. bv/bo fold into one bias row.
"""

import numpy as np

import concourse.bass as bass
import concourse.mybir as mybir
import concourse.tile as tile
from concourse import bacc
from concourse.bass_utils import run_bass_kernel_spmd

B, N, FEAT, H = 4, 1024, 512, 8
HD = FEAT // H  # 64
SCALE = HD ** -0.5
N_CORES = 8
IH = N // 2  # 512 query rows per core
NJT = N // 128  # 8 j-chunks
NCP = NJT // 2  # 4 chunk pairs
NFC = FEAT // 128  # 4 feature chunks

F32 = mybir.dt.float32
F16 = mybir.dt.float16
BF16 = mybir.dt.bfloat16
AF = mybir.ActivationFunctionType

# dtype knobs (numerics vs PE speed; fp32 matmul is 4 cyc/row, 16-bit is 1)
QK_DT = F16   # dtype of QT/KT tiles feeding the score matmuls
PV_DT = F16   # dtype of exp(scores) and V' tiles feeding the PV matmuls
PROJ_DT = F16  # dtype of ndataT/W tiles feeding the QKV projections
FIN_DT = F16  # dtype of normalized head outputs / WoT for the final proj
BIAS_DT = F16  # dtype of the (mask-folded) attention bias in HBM/SBUF
MASK_VAL = -30000.0 if BIAS_DT == F16 else -1e30

_CACHE = {}


def _build():
    nc = bacc.Bacc("TRN2", target_bir_lowering=False, debug=False)

    ndataT = nc.dram_tensor("ndataT", [FEAT, N], PROJ_DT, kind="ExternalInput").ap()
    ndataTq = nc.dram_tensor("ndataTq", [FEAT, IH], PROJ_DT, kind="ExternalInput").ap()
    biasT = nc.dram_tensor("biasT", [H, 128, 4096], BIAS_DT, kind="ExternalInput").ap()
    wqT = nc.dram_tensor("wqT", [FEAT, FEAT], PROJ_DT, kind="ExternalInput").ap()
    wkT = nc.dram_tensor("wkT", [FEAT, FEAT], PROJ_DT, kind="ExternalInput").ap()
    wvT = nc.dram_tensor("wvT", [FEAT, FEAT], PROJ_DT, kind="ExternalInput").ap()
    woT = nc.dram_tensor("woT", [FEAT, FEAT], FIN_DT, kind="ExternalInput").ap()
    bqs = nc.dram_tensor("bqs", [128, NFC], F32, kind="ExternalInput").ap()
    bkp = nc.dram_tensor("bkp", [128, NFC], F32, kind="ExternalInput").ap()
    boe = nc.dram_tensor("boe", [128, FEAT], F32, kind="ExternalInput").ap()
    out = nc.dram_tensor("out", [IH, FEAT], F32, kind="ExternalOutput").ap()

    with tile.TileContext(nc) as tc:
        with (
            tc.tile_pool(name="persist", bufs=1) as persist,
            tc.tile_pool(name="work", bufs=3) as work,
            tc.tile_pool(name="biasp", bufs=2) as biasp,
            tc.tile_pool(name="ps_st", bufs=2, space="PSUM") as ps_st,
            tc.tile_pool(name="ps_out", bufs=2, space="PSUM") as ps_out,
            tc.tile_pool(name="ps_mm", bufs=2, space="PSUM") as ps_mm,
        ):
            KT = [persist.tile([128, N], QK_DT, tag=f"kt{t}", name=f"kt{t}") for t in range(NFC)]
            QT = [persist.tile([128, IH], QK_DT, tag=f"qt{t}", name=f"qt{t}") for t in range(NFC)]
            V = [persist.tile([128, H * 65], PV_DT, tag=f"v{j}", name=f"v{j}") for j in range(NJT)]
            wo = [persist.tile([64, FEAT], FIN_DT, tag=f"wo{h}", name=f"wo{h}") for h in range(H)]
            OTn = [persist.tile([64, IH], FIN_DT, tag=f"otn{h}", name=f"otn{h}") for h in range(H)]
            bq_sb = persist.tile([128, NFC], F32, tag="bq")
            bk_sb = persist.tile([128, NFC], F32, tag="bk")
            boe_sb = persist.tile([128, FEAT], F32, tag="boe")
            ones_sb = persist.tile([128, 64], FIN_DT, tag="ones")

            nc.sync.dma_start(out=bq_sb, in_=bqs)
            nc.sync.dma_start(out=bk_sb, in_=bkp)
            nc.gpsimd.memset(ones_sb, 1.0)

            # ---- phase A: projections ----
            with tc.tile_pool(name="phA", bufs=1) as phA:
                nd, ndq, wq, wk, wv = [], [], [], [], []
                for fc in range(NFC):
                    t = phA.tile([128, N], PROJ_DT, tag=f"nd{fc}")
                    tq = phA.tile([128, IH], PROJ_DT, tag=f"ndq{fc}")
                    nc.sync.dma_start(out=t, in_=ndataT[fc * 128:(fc + 1) * 128, :])
                    nc.sync.dma_start(out=tq, in_=ndataTq[fc * 128:(fc + 1) * 128, :])
                    nd.append(t)
                    ndq.append(tq)
                    for name, lst, src in (("wq", wq, wqT), ("wk", wk, wkT),
                                           ("wv", wv, wvT)):
                        w = phA.tile([128, FEAT], PROJ_DT, tag=f"{name}{fc}")
                        nc.sync.dma_start(out=w, in_=src[fc * 128:(fc + 1) * 128, :])
                        lst.append(w)

                for t in range(NFC):
                    for jh in range(2):
                        ps = ps_mm.tile([128, 512], F32, tag="mm")
                        for fc in range(NFC):
                            nc.tensor.matmul(
                                ps,
                                wk[fc][:, t * 128:(t + 1) * 128],
                                nd[fc][:, jh * 512:(jh + 1) * 512],
                                start=(fc == 0),
                                stop=(fc == NFC - 1),
                            )
                        nc.scalar.activation(
                            KT[t][:, jh * 512:(jh + 1) * 512], ps, AF.Identity,
                            bias=bk_sb[:, t:t + 1],
                        )
                    ps = ps_mm.tile([128, 512], F32, tag="mm")
                    for fc in range(NFC):
                        nc.tensor.matmul(
                            ps,
                            wq[fc][:, t * 128:(t + 1) * 128],
                            ndq[fc],
                            start=(fc == 0),
                            stop=(fc == NFC - 1),
                        )
                    nc.scalar.activation(
                        QT[t], ps, AF.Identity, bias=bq_sb[:, t:t + 1], scale=SCALE,
                    )

                for jt in range(NJT):
                    v = V[jt]
                    nc.gpsimd.memset(
                        v.rearrange("p (h x) -> p h x", x=65)[:, :, 64:65], 1.0
                    )
                    ps = ps_mm.tile([128, 512], F32, tag="mm")
                    for fc in range(NFC):
                        nc.tensor.matmul(
                            ps,
                            nd[fc][:, jt * 128:(jt + 1) * 128],
                            wv[fc],
                            start=(fc == 0),
                            stop=(fc == NFC - 1),
                        )
                    nc.scalar.activation(
                        v.rearrange("p (h x) -> p h x", x=65)[:, :, 0:64],
                        ps.rearrange("p (h x) -> p h x", x=64),
                        AF.Copy,
                    )

            nc.sync.dma_start(out=boe_sb, in_=boe)
            for h in range(H):
                nc.sync.dma_start(out=wo[h], in_=woT[h * 64:(h + 1) * 64, :])

            # ---- phase B: attention, head pairs (row-group packed QK) ----
            for hp in range(H // 2):
                t = hp
                heads = (2 * hp, 2 * hp + 1)
                bias_hp, ssb_hp, et_hp, outp_hp = [], [], [], []
                for idx, h in enumerate(heads):
                    bias_sb = biasp.tile([128, 4096], BIAS_DT, tag="bias",
                                         name=f"bias{h}")
                    nc.sync.dma_start(out=bias_sb, in_=biasT[h])
                    bias_hp.append(bias_sb)
                    ssb_hp.append(work.tile([128, 4096], F32, tag="ssb", bufs=2,
                                            name=f"ssb{h}"))
                    et_hp.append(work.tile([128, 4096], PV_DT, tag="et", bufs=2,
                                           name=f"et{h}"))
                    outp_hp.append(ps_out.tile([65, 512], F32, tag="outp",
                                               name=f"outp{h}"))
                for cp in range(NCP):
                    st_hp = [
                        ps_st.tile([128, 1024], F32, tag=f"st{idx}", bufs=1,
                                   name=f"st{idx}_{hp}_{cp}")
                        for idx in range(2)
                    ]
                    for half in range(2):
                        c = cp * 2 + half
                        for idx in range(2):
                            po = idx * 64
                            nc.tensor.matmul(
                                st_hp[idx][:, half * 512:(half + 1) * 512],
                                KT[t][po:po + 64, c * 128:(c + 1) * 128],
                                QT[t][po:po + 64, :],
                                start=True,
                                stop=True,
                            )
                    for idx in range(2):
                        nc.vector.tensor_add(
                            ssb_hp[idx][:, cp * 1024:(cp + 1) * 1024], st_hp[idx],
                            bias_hp[idx][:, cp * 1024:(cp + 1) * 1024],
                        )
                for idx, h in enumerate(heads):
                    nc.scalar.activation(et_hp[idx], ssb_hp[idx], AF.Exp)
                    for c in range(NJT):
                        nc.tensor.matmul(
                            outp_hp[idx],
                            V[c][:, h * 65:h * 65 + 65],
                            et_hp[idx][:, c * 512:(c + 1) * 512],
                            start=(c == 0),
                            stop=(c == NJT - 1),
                        )
                    # evacuate: rows 0..63 = raw head out, row 64 = denom
                    otd = work.tile([65, IH], F32, tag="otd", name=f"otd{h}")
                    nc.scalar.activation(otd, outp_hp[idx], AF.Copy)
                    rec = work.tile([65, IH], F32, tag="rec", name=f"rec{h}")
                    nc.vector.reciprocal(rec[64:65, :], otd[64:65, :])
                    rec16 = work.tile([65, IH], FIN_DT, tag="rec16",
                                      name=f"rec16{h}")
                    nc.scalar.activation(rec16[64:65, :], rec[64:65, :], AF.Copy)
                    rbc = ps_mm.tile([64, 512], F32, tag="mm", name=f"rbc{h}")
                    nc.tensor.matmul(
                        rbc, ones_sb[64:65, :], rec16[64:65, :], start=True,
                        stop=True,
                    )
                    nc.vector.tensor_mul(OTn[h], otd[0:64, :], rbc)

            # ---- final projection ----
            for it in range(4):
                fp = ps_mm.tile([128, 512], F32, tag="mm")
                for h in range(H):
                    nc.tensor.matmul(
                        fp,
                        OTn[h][:, it * 128:(it + 1) * 128],
                        wo[h],
                        start=(h == 0),
                        stop=(h == H - 1),
                    )
                fsb = work.tile([128, 512], F32, tag="fsb")
                nc.vector.tensor_add(fsb, fp, boe_sb)
                nc.sync.dma_start(out=out[it * 128:(it + 1) * 128, :], in_=fsb)

    nc.compile()
    return nc


def _prep_inputs(ndata, attn_bias, attn_mask, Wq, bq, Wk, bk, Wv, bv, Wo, bo):
    ndata = np.asarray(ndata, dtype=np.float32)
    attn_bias = np.asarray(attn_bias, dtype=np.float32)
    attn_mask = np.asarray(attn_mask)
    Wq, Wk, Wv, Wo = (np.asarray(w, dtype=np.float32) for w in (Wq, Wk, Wv, Wo))
    bq, bk, bv, bo = (np.asarray(v, dtype=np.float32) for v in (bq, bk, bv, bo))

    biasw = np.where(attn_mask, np.float32(MASK_VAL), attn_bias)  # [B, N, N, H]
    np_bias = np.float16 if BIAS_DT == F16 else np.float32

    np_proj = np.float16 if PROJ_DT == F16 else np.float32
    np_fin = np.float16 if FIN_DT == F16 else np.float32
    wqT = np.ascontiguousarray(Wq.T.astype(np_proj))
    wkT = np.ascontiguousarray(Wk.T.astype(np_proj))
    wvT = np.ascontiguousarray(Wv.T.astype(np_proj))
    woT = np.ascontiguousarray(Wo.T.astype(np_fin))
    bqs = np.ascontiguousarray((bq * SCALE).reshape(NFC, 128).T)
    bkp = np.ascontiguousarray(bk.reshape(NFC, 128).T)
    boe = np.tile((bo + bv @ Wo.T)[None, :], (128, 1)).astype(np.float32)

    in_maps = []
    for core in range(N_CORES):
        b, ih = core // 2, core % 2
        i0 = ih * IH
        bw = biasw[b, i0:i0 + IH]  # [512(i), 1024(j), 8(h)]
        arr = bw.transpose(2, 1, 0)  # [8, 1024(j), 512(i)]
        arr = arr.reshape(H, NCP, 2, 128, IH)
        arr = arr.transpose(0, 3, 1, 2, 4)  # [8, 128(p), 4(cp), 2(half), 512]
        arr = np.ascontiguousarray(arr.reshape(H, 128, 4096).astype(np_bias))
        in_maps.append({
            "ndataT": np.ascontiguousarray(ndata[b].T.astype(np_proj)),
            "ndataTq": np.ascontiguousarray(ndata[b, i0:i0 + IH].T.astype(np_proj)),
            "biasT": arr,
            "wqT": wqT, "wkT": wkT, "wvT": wvT, "woT": woT,
            "bqs": bqs, "bkp": bkp, "boe": boe,
        })
    return in_maps


def kernel(ndata, attn_bias, attn_mask, Wq, bq, Wk, bk, Wv, bv, Wo, bo,
           _trace=False):
    if "nc" not in _CACHE:
        _CACHE["nc"] = _build()
    nc = _CACHE["nc"]
    in_maps = _prep_inputs(ndata, attn_bias, attn_mask, Wq, bq, Wk, bk, Wv, bv,
                           Wo, bo)
    res = run_bass_kernel_spmd(nc, in_maps, list(range(N_CORES)), trace=_trace)
    _CACHE["last_res"] = res
    full = np.empty((B, N, FEAT), dtype=np.float32)
    for core in range(N_CORES):
        b, ih = core // 2, core % 2
        full[b, ih * IH:(ih + 1) * IH, :] = res.results[core]["out"]
    return full


# revision 11
# speedup vs baseline: 1.1470x; 1.1470x over previous
"""BiasedMHA Trainium2 kernel.

Problem: B=4, N=1024, FEAT=512, H=8 multihead attention with additive bias and
boolean mask, softmax over the key dim, output projection.

Sharding (8 cores): core c handles batch b = c//2 and query-row half
ih = c%2 (512 query rows). Each core computes the full attention + output
projection for its slice -> out[b, ih*512:(ih+1)*512, :]. No collectives.

Per-core device layout (everything pre-transposed on host so DMAs are
contiguous and matmuls contract along partitions):
  - QT[d, i], KT[d, j] (head-major d), V'[j, 8*(64+1)] with a ones column per
    head (gives the softmax denominator for free during the PV matmul).
  - scores computed transposed: ST[j-chunk, i] = K_h^T(chunk) x Q_h
  - bias (with mask folded in as -1e30) is added with the DVE, exp on ACT.
  - PV: out'[65, i] accumulated over j-chunks; row 64 = sum of exp = denom.
  - normalization: recip = 1/denom, broadcast along partitions via a K=1
    matmul, applied with one tensor-tensor multiply; the output projection
    then runs on normalized head outputs. bv/bo fold into one bias row.
"""

import numpy as np

import concourse.bass as bass
import concourse.mybir as mybir
import concourse.tile as tile
from concourse import bacc
from concourse.bass_utils import run_bass_kernel_spmd

B, N, FEAT, H = 4, 1024, 512, 8
HD = FEAT // H  # 64
SCALE = HD ** -0.5
N_CORES = 8
IH = N // 2  # 512 query rows per core
NJT = N // 128  # 8 j-chunks
NCP = NJT // 2  # 4 chunk pairs
NFC = FEAT // 128  # 4 feature chunks

F32 = mybir.dt.float32
F16 = mybir.dt.float16
BF16 = mybir.dt.bfloat16
AF = mybir.ActivationFunctionType

# dtype knobs (numerics vs PE speed; fp32 matmul is 4 cyc/row, 16-bit is 1)
QK_DT = F16   # dtype of QT/KT tiles feeding the score matmuls
PV_DT = F16   # dtype of exp(scores) and V' tiles feeding the PV matmuls
PROJ_DT = F16  # dtype of ndataT/W tiles feeding the QKV projections
FIN_DT = F16  # dtype of normalized head outputs / WoT for the final proj
BIAS_DT = F16  # dtype of the (mask-folded) attention bias in HBM/SBUF
MASK_VAL = -30000.0 if BIAS_DT == F16 else -1e30

_CACHE = {}


def _build():
    nc = bacc.Bacc("TRN2", target_bir_lowering=False, debug=False)

    ndataT = nc.dram_tensor("ndataT", [FEAT, N], PROJ_DT, kind="ExternalInput").ap()
    ndataTq = nc.dram_tensor("ndataTq", [FEAT, IH], PROJ_DT, kind="ExternalInput").ap()
    biasT = nc.dram_tensor("biasT", [H, 128, 4096], BIAS_DT, kind="ExternalInput").ap()
    wqT = nc.dram_tensor("wqT", [FEAT, FEAT], PROJ_DT, kind="ExternalInput").ap()
    wkT = nc.dram_tensor("wkT", [FEAT, FEAT], PROJ_DT, kind="ExternalInput").ap()
    wvT = nc.dram_tensor("wvT", [FEAT, FEAT], PROJ_DT, kind="ExternalInput").ap()
    woT = nc.dram_tensor("woT", [FEAT, FEAT], FIN_DT, kind="ExternalInput").ap()
    bqs = nc.dram_tensor("bqs", [128, NFC], F32, kind="ExternalInput").ap()
    bkp = nc.dram_tensor("bkp", [128, NFC], F32, kind="ExternalInput").ap()
    boe = nc.dram_tensor("boe", [128, FEAT], F32, kind="ExternalInput").ap()
    out = nc.dram_tensor("out", [IH, FEAT], F32, kind="ExternalOutput").ap()

    with tile.TileContext(nc) as tc:
        with (
            tc.tile_pool(name="persist", bufs=1) as persist,
            tc.tile_pool(name="work", bufs=3) as work,
            tc.tile_pool(name="biasp", bufs=2) as biasp,
            tc.tile_pool(name="ps_st", bufs=2, space="PSUM") as ps_st,
            tc.tile_pool(name="ps_out", bufs=2, space="PSUM") as ps_out,
            tc.tile_pool(name="ps_mm", bufs=2, space="PSUM") as ps_mm,
        ):
            KT = [persist.tile([128, N], QK_DT, tag=f"kt{t}", name=f"kt{t}") for t in range(NFC)]
            QT = [persist.tile([128, IH], QK_DT, tag=f"qt{t}", name=f"qt{t}") for t in range(NFC)]
            V = [persist.tile([128, H * 65], PV_DT, tag=f"v{j}", name=f"v{j}") for j in range(NJT)]
            wo = [persist.tile([64, FEAT], FIN_DT, tag=f"wo{h}", name=f"wo{h}") for h in range(H)]
            OTn = [persist.tile([64, IH], FIN_DT, tag=f"otn{h}", name=f"otn{h}") for h in range(H)]
            bq_sb = persist.tile([128, NFC], F32, tag="bq")
            bk_sb = persist.tile([128, NFC], F32, tag="bk")
            boe_sb = persist.tile([128, FEAT], F32, tag="boe")
            ones_sb = persist.tile([128, 64], FIN_DT, tag="ones")

            nc.sync.dma_start(out=bq_sb, in_=bqs)
            nc.sync.dma_start(out=bk_sb, in_=bkp)
            nc.gpsimd.memset(ones_sb, 1.0)

            # ---- phase A: projections ----
            with tc.tile_pool(name="phA", bufs=1) as phA:
                nd, ndq, wq, wk, wv = [], [], [], [], []
                for fc in range(NFC):
                    t = phA.tile([128, N], PROJ_DT, tag=f"nd{fc}")
                    tq = phA.tile([128, IH], PROJ_DT, tag=f"ndq{fc}")
                    nc.sync.dma_start(out=t, in_=ndataT[fc * 128:(fc + 1) * 128, :])
                    nc.sync.dma_start(out=tq, in_=ndataTq[fc * 128:(fc + 1) * 128, :])
                    nd.append(t)
                    ndq.append(tq)
                    for name, lst, src in (("wq", wq, wqT), ("wk", wk, wkT),
                                           ("wv", wv, wvT)):
                        w = phA.tile([128, FEAT], PROJ_DT, tag=f"{name}{fc}")
                        nc.sync.dma_start(out=w, in_=src[fc * 128:(fc + 1) * 128, :])
                        lst.append(w)

                for t in range(NFC):
                    for jh in range(2):
                        ps = ps_mm.tile([128, 512], F32, tag="mm")
                        for fc in range(NFC):
                            nc.tensor.matmul(
                                ps,
                                wk[fc][:, t * 128:(t + 1) * 128],
                                nd[fc][:, jh * 512:(jh + 1) * 512],
                                start=(fc == 0),
                                stop=(fc == NFC - 1),
                            )
                        nc.scalar.activation(
                            KT[t][:, jh * 512:(jh + 1) * 512], ps, AF.Identity,
                            bias=bk_sb[:, t:t + 1],
                        )
                    ps = ps_mm.tile([128, 512], F32, tag="mm")
                    for fc in range(NFC):
                        nc.tensor.matmul(
                            ps,
                            wq[fc][:, t * 128:(t + 1) * 128],
                            ndq[fc],
                            start=(fc == 0),
                            stop=(fc == NFC - 1),
                        )
                    nc.scalar.activation(
                        QT[t], ps, AF.Identity, bias=bq_sb[:, t:t + 1], scale=SCALE,
                    )

                for jt in range(NJT):
                    v = V[jt]
                    nc.gpsimd.memset(
                        v.rearrange("p (h x) -> p h x", x=65)[:, :, 64:65], 1.0
                    )
                    ps = ps_mm.tile([128, 512], F32, tag="mm")
                    for fc in range(NFC):
                        nc.tensor.matmul(
                            ps,
                            nd[fc][:, jt * 128:(jt + 1) * 128],
                            wv[fc],
                            start=(fc == 0),
                            stop=(fc == NFC - 1),
                        )
                    nc.scalar.activation(
                        v.rearrange("p (h x) -> p h x", x=65)[:, :, 0:64],
                        ps.rearrange("p (h x) -> p h x", x=64),
                        AF.Copy,
                    )

            nc.sync.dma_start(out=boe_sb, in_=boe)
            for h in range(H):
                nc.sync.dma_start(out=wo[h], in_=woT[h * 64:(h + 1) * 64, :])

            # ---- phase B: attention, head pairs (row-group packed QK) ----
            for hp in range(H // 2):
                t = hp
                heads = (2 * hp, 2 * hp + 1)
                bias_hp, ssb_hp, et_hp, outp_hp = [], [], [], []
                for idx, h in enumerate(heads):
                    bias_sb = biasp.tile([128, 4096], BIAS_DT, tag="bias", bufs=4,
                                         name=f"bias{h}")
                    nc.sync.dma_start(out=bias_sb, in_=biasT[h])
                    bias_hp.append(bias_sb)
                    ssb_hp.append(work.tile([128, 4096], F32, tag="ssb", bufs=3,
                                            name=f"ssb{h}"))
                    et_hp.append(work.tile([128, 4096], PV_DT, tag="et", bufs=3,
                                           name=f"et{h}"))
                    outp_hp.append(ps_out.tile([65, 512], F32, tag="outp",
                                               name=f"outp{h}"))
                for cp in range(NCP):
                    for half in range(2):
                        c = cp * 2 + half
                        for idx in range(2):
                            po = idx * 64
                            st = ps_st.tile([128, 512], F32, tag="st", bufs=4,
                                            name=f"st{hp}_{c}_{idx}")
                            nc.tensor.matmul(
                                st,
                                KT[t][po:po + 64, c * 128:(c + 1) * 128],
                                QT[t][po:po + 64, :],
                                start=True,
                                stop=True,
                            )
                            nc.vector.tensor_add(
                                ssb_hp[idx][:, c * 512:(c + 1) * 512], st,
                                bias_hp[idx][:, c * 512:(c + 1) * 512],
                            )
                for idx, h in enumerate(heads):
                    nc.scalar.activation(et_hp[idx], ssb_hp[idx], AF.Exp)
                    for c in range(NJT):
                        nc.tensor.matmul(
                            outp_hp[idx],
                            V[c][:, h * 65:h * 65 + 65],
                            et_hp[idx][:, c * 512:(c + 1) * 512],
                            start=(c == 0),
                            stop=(c == NJT - 1),
                        )
                    # evacuate: rows 0..63 = raw head out, row 64 = denom
                    otd = work.tile([65, IH], F32, tag="otd", name=f"otd{h}")
                    nc.scalar.activation(otd, outp_hp[idx], AF.Copy)
                    lnv = work.tile([65, IH], F32, tag="lnv", name=f"lnv{h}")
                    nc.scalar.activation(lnv[64:65, :], otd[64:65, :], AF.Ln)
                    rec16 = work.tile([65, IH], FIN_DT, tag="rec16",
                                      name=f"rec16{h}")
                    nc.scalar.activation(rec16[64:65, :], lnv[64:65, :], AF.Exp,
                                         scale=-1.0)
                    rbc = ps_mm.tile([64, 512], F32, tag="mm", name=f"rbc{h}")
                    nc.tensor.matmul(
                        rbc, ones_sb[64:65, :], rec16[64:65, :], start=True,
                        stop=True,
                    )
                    nc.vector.tensor_mul(OTn[h], otd[0:64, :], rbc)

            # ---- final projection ----
            for it in range(4):
                fp = ps_mm.tile([128, 512], F32, tag="mm")
                for h in range(H):
                    nc.tensor.matmul(
                        fp,
                        OTn[h][:, it * 128:(it + 1) * 128],
                        wo[h],
                        start=(h == 0),
                        stop=(h == H - 1),
                    )
                fsb = work.tile([128, 512], F32, tag="fsb")
                nc.vector.tensor_add(fsb, fp, boe_sb)
                nc.sync.dma_start(out=out[it * 128:(it + 1) * 128, :], in_=fsb)

    nc.compile()
    return nc


def _prep_inputs(ndata, attn_bias, attn_mask, Wq, bq, Wk, bk, Wv, bv, Wo, bo):
    ndata = np.asarray(ndata, dtype=np.float32)
    attn_bias = np.asarray(attn_bias, dtype=np.float32)
    attn_mask = np.asarray(attn_mask)
    Wq, Wk, Wv, Wo = (np.asarray(w, dtype=np.float32) for w in (Wq, Wk, Wv, Wo))
    bq, bk, bv, bo = (np.asarray(v, dtype=np.float32) for v in (bq, bk, bv, bo))

    biasw = np.where(attn_mask, np.float32(MASK_VAL), attn_bias)  # [B, N, N, H]
    np_bias = np.float16 if BIAS_DT == F16 else np.float32

    np_proj = np.float16 if PROJ_DT == F16 else np.float32
    np_fin = np.float16 if FIN_DT == F16 else np.float32
    wqT = np.ascontiguousarray(Wq.T.astype(np_proj))
    wkT = np.ascontiguousarray(Wk.T.astype(np_proj))
    wvT = np.ascontiguousarray(Wv.T.astype(np_proj))
    woT = np.ascontiguousarray(Wo.T.astype(np_fin))
    bqs = np.ascontiguousarray((bq * SCALE).reshape(NFC, 128).T)
    bkp = np.ascontiguousarray(bk.reshape(NFC, 128).T)
    boe = np.tile((bo + bv @ Wo.T)[None, :], (128, 1)).astype(np.float32)

    in_maps = []
    for core in range(N_CORES):
        b, ih = core // 2, core % 2
        i0 = ih * IH
        bw = biasw[b, i0:i0 + IH]  # [512(i), 1024(j), 8(h)]
        arr = bw.transpose(2, 1, 0)  # [8, 1024(j), 512(i)]
        arr = arr.reshape(H, NCP, 2, 128, IH)
        arr = arr.transpose(0, 3, 1, 2, 4)  # [8, 128(p), 4(cp), 2(half), 512]
        arr = np.ascontiguousarray(arr.reshape(H, 128, 4096).astype(np_bias))
        in_maps.append({
            "ndataT": np.ascontiguousarray(ndata[b].T.astype(np_proj)),
            "ndataTq": np.ascontiguousarray(ndata[b, i0:i0 + IH].T.astype(np_proj)),
            "biasT": arr,
            "wqT": wqT, "wkT": wkT, "wvT": wvT, "woT": woT,
            "bqs": bqs, "bkp": bkp, "boe": boe,
        })
    return in_maps


def kernel(ndata, attn_bias, attn_mask, Wq, bq, Wk, bk, Wv, bv, Wo, bo,
           _trace=False):
    if "nc" not in _CACHE:
        _CACHE["nc"] = _build()
    nc = _CACHE["nc"]
    in_maps = _prep_inputs(ndata, attn_bias, attn_mask, Wq, bq, Wk, bk, Wv, bv,
                           Wo, bo)
    res = run_bass_kernel_spmd(nc, in_maps, list(range(N_CORES)), trace=_trace)
    _CACHE["last_res"] = res
    full = np.empty((B, N, FEAT), dtype=np.float32)
    for core in range(N_CORES):
        b, ih = core // 2, core % 2
        full[b, ih * IH:(ih + 1) * IH, :] = res.results[core]["out"]
    return full
